# revision 1
# baseline (speedup 1.0000x reference)
"""GRU cell kernel for Trainium2, data-parallel across 8 NeuronCores.

Per core: batch shard of 1024 rows; weights replicated.
  u  = sigmoid(x @ Wxu + h @ Whu + bu)
  r  = sigmoid(x @ Wxr + h @ Whr + br)
  c' = tanh  (x @ Wxc + (h*r) @ Whc + bc)
  c  = u*c' + (1-u)*h

Layout: all activations kept transposed in SBUF ([feature, batch]) so the
contraction dim lands on partitions; weights load in natural layout as the
stationary operand; matmuls run in float32r (full PE rate at 512-col moving).
x/h are transposed on entry and c back on exit via PE transposes.
"""

import os
import sys

import numpy as np

B = 8192
E = 1024
H = 1024
NCORES = 8
B_SH = B // NCORES  # 1024 rows per core

P = 128
KE = E // P   # 8 contraction chunks for x-side
KH = H // P   # 8 contraction chunks for h-side
NJ = H // P   # 8 output feature chunks
BN = 512      # moving free-dim per matmul (fp32 max)
NB = B_SH // BN  # 2

W_NAMES = ("Wxu", "Whu", "Wxr", "Whr", "Wxc", "Whc")
B_NAMES = ("bu", "br", "bc")

_NC_CACHE = {}


def _ensure_paths():
    for p in ("/opt/trn_rl_repo", "/root/.axon_site/_ro/trn_rl_repo"):
        if os.path.isdir(p) and p not in sys.path:
            sys.path.insert(0, p)


def _build_nc():
    import concourse.bass as bass
    import concourse.mybir as mybir
    from concourse.masks import make_identity
    from concourse.tile import TileContext

    f32 = mybir.dt.float32
    bf16 = mybir.dt.bfloat16
    AF = mybir.ActivationFunctionType

    nc = bass.Bass()
    x_d = nc.dram_tensor("input", [B_SH, E], f32, kind="ExternalInput")
    h_d = nc.dram_tensor("hidden_state", [B_SH, H], f32, kind="ExternalInput")
    w_d = {n: nc.dram_tensor(n, [E, H], f32, kind="ExternalInput") for n in W_NAMES}
    b_d = {n: nc.dram_tensor(n, [1, H], f32, kind="ExternalInput") for n in B_NAMES}
    out_d = nc.dram_tensor("output", [B_SH, H], f32, kind="ExternalOutput")

    with TileContext(nc) as tc:
        with (
            tc.tile_pool(name="sb", bufs=1) as sb,
            tc.tile_pool(name="psum", bufs=1, space="PSUM") as pp,
        ):
            ident = sb.tile([P, P], f32, tag="ident", bufs=1)
            make_identity(nc, ident[:])

            xT = [sb.tile([P, B_SH], bf16, tag=f"xT{k}", name=f"xT{k}", bufs=1) for k in range(KE)]
            hT = [sb.tile([P, B_SH], bf16, tag=f"hT{k}", name=f"hT{k}", bufs=1) for k in range(KH)]
            uT = [sb.tile([P, B_SH], f32, tag=f"uT{j}", name=f"uT{j}", bufs=1) for j in range(NJ)]
            rhT = [sb.tile([P, B_SH], bf16, tag=f"rhT{j}", name=f"rhT{j}", bufs=1) for j in range(NJ)]
            hT32 = [sb.tile([P, B_SH], f32, tag=f"hT32{k}", name=f"hT32{k}", bufs=1) for k in range(KH)]

            # ---- load x, h and transpose into [feature, batch] layout ----
            for src_d, dstT in ((x_d, xT), (h_d, hT)):
                for bi in range(B_SH // P):
                    nat = sb.tile([P, E], f32, tag="nat", bufs=3)
                    nc.sync.dma_start(nat[:], src_d[bi * P : (bi + 1) * P, :])
                    for ej in range(KE):
                        ps = pp.tile([P, P], f32, tag="pstr", bufs=4)
                        nc.tensor.transpose(
                            ps[:], nat[:, ej * P : (ej + 1) * P], ident[:]
                        )
                        nc.vector.tensor_copy(
                            dstT[ej][:, bi * P : (bi + 1) * P], ps[:]
                        )
                        if dstT is hT:
                            nc.vector.tensor_copy(
                                hT32[ej][:, bi * P : (bi + 1) * P], ps[:]
                            )

            bias_t = {}
            for g, nm in (("u", "bu"), ("r", "br"), ("c", "bc")):
                bt = sb.tile([P, NJ], f32, tag=f"bias_{g}", bufs=1)
                for j in range(NJ):
                    nc.sync.dma_start(
                        bt[:, j : j + 1],
                        b_d[nm][0:1, j * P : (j + 1) * P].rearrange("a p -> p a"),
                    )
                bias_t[g] = bt

            def dma_w_ktiles(wname):
                tiles = []
                for k in range(KE):
                    ws = sb.tile([P, E], f32, tag="wstage", name=f"ws_{wname}_{k}", bufs=3)
                    nc.sync.dma_start(ws[:], w_d[wname][k * P : (k + 1) * P, :])
                    wt = sb.tile([P, E], bf16, tag="w", name=f"w_{wname}_{k}", bufs=18)
                    nc.vector.tensor_copy(wt[:], ws[:])
                    tiles.append(wt)
                return tiles

            def gate_matmuls(ps, wxs, whs, rhsT, j, n):
                jsl = slice(j * P, (j + 1) * P)
                sl = slice(n * BN, (n + 1) * BN)
                for k in range(KE):
                    nc.tensor.matmul(
                        ps[:],
                        wxs[k][:, jsl],
                        xT[k][:, sl],
                        start=(k == 0),
                        stop=False,
                    )
                for k in range(KH):
                    nc.tensor.matmul(
                        ps[:],
                        whs[k][:, jsl],
                        rhsT[k][:, sl],
                        start=False,
                        stop=(k == KH - 1),
                    )

            # ---- gate r, then u (both sigmoid); r is folded into r*h ----
            for gname, wx, wh, dst in (("r", "Wxr", "Whr", rhT), ("u", "Wxu", "Whu", uT)):
                wxs = dma_w_ktiles(wx)
                whs = dma_w_ktiles(wh)
                for j in range(NJ):
                    for n in range(NB):
                        sl = slice(n * BN, (n + 1) * BN)
                        ps = pp.tile([P, BN], f32, tag="mm", bufs=4)
                        gate_matmuls(ps, wxs, whs, hT, j, n)
                        nc.scalar.activation(
                            dst[j][:, sl], ps[:], AF.Sigmoid,
                            bias=bias_t[gname][:, j : j + 1],
                        )
                        if gname == "r":
                            nc.vector.tensor_mul(
                                dst[j][:, sl], dst[j][:, sl], hT[j][:, sl]
                            )

            # ---- candidate + blend + output transpose, per feature chunk ----
            wxs = dma_w_ktiles("Wxc")
            whs = dma_w_ktiles("Whc")
            for j in range(NJ):
                cc = sb.tile([P, B_SH], f32, tag="cc", bufs=3)
                for n in range(NB):
                    sl = slice(n * BN, (n + 1) * BN)
                    ps = pp.tile([P, BN], f32, tag="mm", bufs=4)
                    gate_matmuls(ps, wxs, whs, rhT, j, n)
                    nc.scalar.activation(
                        cc[:, sl], ps[:], AF.Tanh, bias=bias_t["c"][:, j : j + 1]
                    )
                    # c = h + u*(c' - h), computed in place in cc
                    nc.vector.tensor_sub(cc[:, sl], cc[:, sl], hT32[j][:, sl])
                    nc.vector.tensor_mul(cc[:, sl], cc[:, sl], uT[j][:, sl])
                    nc.vector.tensor_add(cc[:, sl], cc[:, sl], hT32[j][:, sl])
                for bi in range(B_SH // P):
                    ps = pp.tile([P, P], f32, tag="pstr", bufs=4)
                    nc.tensor.transpose(
                        ps[:], cc[:, bi * P : (bi + 1) * P], ident[:]
                    )
                    ot = sb.tile([P, P], f32, tag="ost", bufs=4)
                    nc.vector.tensor_copy(ot[:], ps[:])
                    nc.sync.dma_start(
                        out_d[bi * P : (bi + 1) * P, j * P : (j + 1) * P], ot[:]
                    )

    _split_matmul_waits(nc, mybir)
    return nc


def _split_matmul_waits(nc, mybir):
    """Walrus codegen allows only one sync-wait on a Matmult (it lowers to an
    LDW+MM pair).  Spill extra waits onto a PE NoOp placed just before."""
    n_fixed = 0
    blocks = list(nc.m.functions[0].blocks)
    origs = [list(b.instructions) for b in blocks]
    spill_nops = {}  # id(inst) -> [nop insts]
    for orig in origs:
        for inst in orig:
            si = inst.sync_info
            if (
                si is not None
                and si.on_wait
                and len(si.on_wait) > 1
            ):
                waits = list(si.on_wait)
                eng = nc.engines[inst.engine]
                nops = []
                for w in waits[:-1]:
                    nop = eng.nop(hint="waitspill").ins
                    nop.sync_info = mybir.SyncInfo(on_wait=[w], on_update=[])
                    nops.append(nop)
                inst.sync_info = mybir.SyncInfo(
                    on_wait=waits[-1:], on_update=list(si.on_update or [])
                )
                spill_nops[id(inst)] = nops
                n_fixed += 1
    for blk, orig in zip(blocks, origs):
        new_list = []
        for inst in orig:
            if id(inst) in spill_nops:
                new_list.extend(spill_nops[id(inst)])
            new_list.append(inst)
        # rebuilding from `orig` also drops any freshly created nops that
        # bass appended to this block's tail
        blk.instructions[:] = new_list
    return n_fixed


def get_nc():
    if "nc" not in _NC_CACHE:
        _ensure_paths()
        _NC_CACHE["nc"] = _build_nc()
    return _NC_CACHE["nc"]


def kernel(**inputs):
    _ensure_paths()
    from concourse.bass_utils import run_bass_kernel_spmd

    nc = get_nc()

    x = np.ascontiguousarray(np.asarray(inputs["input"], dtype=np.float32))
    h = np.ascontiguousarray(np.asarray(inputs["hidden_state"], dtype=np.float32))
    shared = {
        n: np.ascontiguousarray(np.asarray(inputs[n], dtype=np.float32))
        for n in W_NAMES + B_NAMES
    }
    in_maps = []
    for c in range(NCORES):
        m = {
            "input": x[c * B_SH : (c + 1) * B_SH],
            "hidden_state": h[c * B_SH : (c + 1) * B_SH],
        }
        m.update(shared)
        in_maps.append(m)

    res = run_bass_kernel_spmd(nc, in_maps, list(range(NCORES)))
    out = np.concatenate(
        [np.asarray(res.results[c]["output"]) for c in range(NCORES)], axis=0
    )
    return out.astype(np.float32)



# revision 4
# speedup vs baseline: 1.4859x; 1.4859x over previous
"""GRU cell kernel for Trainium2, data-parallel across 8 NeuronCores.

Per core: batch shard of 1024 rows; weights replicated.
  u  = sigmoid(x @ Wxu + h @ Whu + bu)
  r  = sigmoid(x @ Wxr + h @ Whr + br)
  c' = tanh  (x @ Wxc + (h*r) @ Whc + bc)
  c  = u*c' + (1-u)*h

All layout work happens on the HOST: activations are fed pre-transposed
([feature, batch]) and pre-packed in the exact SBUF column order, weights
pre-converted to bf16 in stationary-operand order, and the output comes
back feature-major and is unscrambled host-side.  The device therefore
runs nothing but the 768 bf16 matmuls (128x128x512 each), the three
activations, and the blend -- no on-chip transposes, no dtype casts.

SBUF column layouts (all tiles [128 partitions x cols]):
  activations x,h : col = n*4096 + k*512 + b   (n: batch half, k: feat chunk)
  weights         : col = j*1024 + k*128 + jc  (j: out chunk, k: contraction)
  output          : col = j*1024 + n*512 + b
"""

import os
import sys

import numpy as np

B = 8192
E = 1024
H = 1024
NCORES = 8
B_SH = B // NCORES  # 1024 rows per core

P = 128
KE = E // P   # 8 contraction chunks
NJ = H // P   # 8 output feature chunks
BN = 512      # moving free-dim per matmul
NB = B_SH // BN  # 2

W_NAMES = ("Wxu", "Whu", "Wxr", "Whr", "Wxc", "Whc")
B_NAMES = ("bu", "br", "bc")

_NC_CACHE = {}


def _ensure_paths():
    for p in ("/opt/trn_rl_repo", "/root/.axon_site/_ro/trn_rl_repo"):
        if os.path.isdir(p) and p not in sys.path:
            sys.path.insert(0, p)


def _bf16():
    import ml_dtypes

    return ml_dtypes.bfloat16


def pack_act(a):
    """[B_SH, 1024] f32 -> [128, 8192] bf16, col = n*4096 + k*512 + b."""
    t = np.asarray(a, np.float32).astype(_bf16()).T          # [feat, batch]
    t = t.reshape(KE, P, NB, BN)                             # [k, p, n, b]
    return np.ascontiguousarray(t.transpose(1, 2, 0, 3).reshape(P, NB * KE * BN))


def pack_w(w):
    """[1024, 1024] f32 -> [128, 8192] bf16, col = j*1024 + k*128 + jc."""
    t = np.asarray(w, np.float32).astype(_bf16()).reshape(KE, P, NJ, P)
    return np.ascontiguousarray(t.transpose(1, 2, 0, 3).reshape(P, NJ * KE * P))


def pack_bias(b):
    """[1, 1024] f32 -> [128, 8] f32, col j holds features j*128..j*128+127."""
    return np.ascontiguousarray(
        np.asarray(b, np.float32).reshape(NJ, P).T.astype(np.float32)
    )


def decode_out(o):
    """[128, 8192] f32 (p; j,n,b cols) -> [B_SH, 1024] f32 [batch, feature]."""
    return np.ascontiguousarray(
        np.asarray(o).reshape(P, NJ, NB, BN).transpose(2, 3, 1, 0).reshape(B_SH, H)
    )


def build_in_maps(inputs):
    x = np.asarray(inputs["input"], dtype=np.float32)
    h = np.asarray(inputs["hidden_state"], dtype=np.float32)
    shared = {n: pack_w(inputs[n]) for n in W_NAMES}
    shared.update({n: pack_bias(inputs[n]) for n in B_NAMES})
    in_maps = []
    for c in range(NCORES):
        m = {
            "input": pack_act(x[c * B_SH : (c + 1) * B_SH]),
            "hidden_state": pack_act(h[c * B_SH : (c + 1) * B_SH]),
        }
        m.update(shared)
        in_maps.append(m)
    return in_maps


def _build_nc():
    import concourse.bass as bass
    import concourse.mybir as mybir
    from concourse.tile import TileContext

    f32 = mybir.dt.float32
    bf16 = mybir.dt.bfloat16
    AF = mybir.ActivationFunctionType

    ACT_COLS = NB * KE * BN  # 8192
    W_COLS = NJ * KE * P     # 8192

    nc = bass.Bass()
    x_d = nc.dram_tensor("input", [P, ACT_COLS], bf16, kind="ExternalInput")
    h_d = nc.dram_tensor("hidden_state", [P, ACT_COLS], bf16, kind="ExternalInput")
    w_d = {n: nc.dram_tensor(n, [P, W_COLS], bf16, kind="ExternalInput") for n in W_NAMES}
    b_d = {n: nc.dram_tensor(n, [P, NJ], f32, kind="ExternalInput") for n in B_NAMES}
    out_d = nc.dram_tensor("output", [P, NJ * B_SH], f32, kind="ExternalOutput")

    with TileContext(nc) as tc:
        with (
            tc.tile_pool(name="sb", bufs=1) as sb,
            tc.tile_pool(name="psum", bufs=1, space="PSUM") as pp,
        ):
            xs = sb.tile([P, ACT_COLS], bf16, tag="xs", bufs=1)
            hs = sb.tile([P, ACT_COLS], bf16, tag="hs", bufs=1)
            us = sb.tile([P, ACT_COLS], bf16, tag="us", bufs=1)
            rhs = sb.tile([P, ACT_COLS], bf16, tag="rhs", bufs=1)
            ws = {
                n: sb.tile([P, W_COLS], bf16, tag=f"w_{n}", name=f"w_{n}", bufs=1)
                for n in W_NAMES
            }
            bt = {
                n: sb.tile([P, NJ], f32, tag=f"b_{n}", name=f"b_{n}", bufs=1)
                for n in B_NAMES
            }

            HALF = ACT_COLS // 2

            # ---- input DMAs, ordered so gate r's first groups unblock early
            nc.sync.dma_start(ws["Wxr"][:, 0:HALF], w_d["Wxr"][:, 0:HALF])
            nc.sync.dma_start(xs[:, 0:HALF], x_d[:, 0:HALF])
            nc.sync.dma_start(ws["Whr"][:, 0:HALF], w_d["Whr"][:, 0:HALF])
            nc.sync.dma_start(hs[:, 0:HALF], h_d[:, 0:HALF])
            for n in B_NAMES:
                nc.sync.dma_start(bt[n][:, :], b_d[n][:, :])
            nc.sync.dma_start(xs[:, HALF:], x_d[:, HALF:])
            nc.sync.dma_start(hs[:, HALF:], h_d[:, HALF:])
            nc.sync.dma_start(ws["Wxr"][:, HALF:], w_d["Wxr"][:, HALF:])
            nc.sync.dma_start(ws["Whr"][:, HALF:], w_d["Whr"][:, HALF:])
            for wn in ("Wxu", "Whu", "Wxc", "Whc"):
                nc.sync.dma_start(ws[wn][:, 0:HALF], w_d[wn][:, 0:HALF])
                nc.sync.dma_start(ws[wn][:, HALF:], w_d[wn][:, HALF:])

            def acc_group(ps, wx, wh, rhsT, j, n):
                """16 matmuls accumulating x-side then h-side into ps."""
                for k in range(KE):
                    nc.tensor.matmul(
                        ps[:],
                        wx[:, j * (KE * P) + k * P : j * (KE * P) + (k + 1) * P],
                        xs[:, n * (KE * BN) + k * BN : n * (KE * BN) + (k + 1) * BN],
                        start=(k == 0),
                        stop=False,
                    )
                for k in range(KE):
                    nc.tensor.matmul(
                        ps[:],
                        wh[:, j * (KE * P) + k * P : j * (KE * P) + (k + 1) * P],
                        rhsT[:, n * (KE * BN) + k * BN : n * (KE * BN) + (k + 1) * BN],
                        start=False,
                        stop=(k == KE - 1),
                    )

            def act_slice(t, j, n):
                return t[:, n * (KE * BN) + j * BN : n * (KE * BN) + (j + 1) * BN]

            # ---- gate r: sigmoid, folded into r*h (bf16, feature-major)
            for j in range(NJ):
                for n in range(NB):
                    ps = pp.tile([P, BN], f32, tag="mm", bufs=6)
                    acc_group(ps, ws["Wxr"], ws["Whr"], hs, j, n)
                    rt = sb.tile([P, BN], bf16, tag="rt", bufs=3)
                    nc.scalar.activation(
                        rt[:], ps[:], AF.Sigmoid, bias=bt["br"][:, j : j + 1]
                    )
                    nc.vector.tensor_mul(
                        act_slice(rhs, j, n), rt[:], act_slice(hs, j, n)
                    )

            # ---- gate u: sigmoid -> us (bf16)
            for j in range(NJ):
                for n in range(NB):
                    ps = pp.tile([P, BN], f32, tag="mm", bufs=6)
                    acc_group(ps, ws["Wxu"], ws["Whu"], hs, j, n)
                    nc.scalar.activation(
                        act_slice(us, j, n), ps[:], AF.Sigmoid,
                        bias=bt["bu"][:, j : j + 1],
                    )

            # ---- candidate + blend + store
            for j in range(NJ):
                for n in range(NB):
                    ps = pp.tile([P, BN], f32, tag="mm", bufs=6)
                    acc_group(ps, ws["Wxc"], ws["Whc"], rhs, j, n)
                    cc = sb.tile([P, BN], f32, tag="cc", bufs=3)
                    nc.scalar.activation(
                        cc[:], ps[:], AF.Tanh, bias=bt["bc"][:, j : j + 1]
                    )
                    # c = h + u*(c' - h)
                    nc.vector.tensor_sub(cc[:], cc[:], act_slice(hs, j, n))
                    nc.vector.tensor_mul(cc[:], cc[:], act_slice(us, j, n))
                    nc.vector.tensor_add(cc[:], cc[:], act_slice(hs, j, n))
                    nc.gpsimd.dma_start(
                        out_d[:, j * B_SH + n * BN : j * B_SH + (n + 1) * BN], cc[:]
                    )

    _split_matmul_waits(nc, mybir)
    return nc


def _split_matmul_waits(nc, mybir):
    """Walrus codegen allows only one sync-wait on a Matmult (it lowers to an
    LDW+MM pair).  Spill extra waits onto a PE NoOp placed just before."""
    n_fixed = 0
    blocks = list(nc.m.functions[0].blocks)
    origs = [list(b.instructions) for b in blocks]
    spill_nops = {}  # id(inst) -> [nop insts]
    for orig in origs:
        for inst in orig:
            si = inst.sync_info
            if (
                si is not None
                and si.on_wait
                and len(si.on_wait) > 1
            ):
                waits = list(si.on_wait)
                eng = nc.engines[inst.engine]
                nops = []
                for w in waits[:-1]:
                    nop = eng.nop(hint="waitspill").ins
                    nop.sync_info = mybir.SyncInfo(on_wait=[w], on_update=[])
                    nops.append(nop)
                inst.sync_info = mybir.SyncInfo(
                    on_wait=waits[-1:], on_update=list(si.on_update or [])
                )
                spill_nops[id(inst)] = nops
                n_fixed += 1
    for blk, orig in zip(blocks, origs):
        new_list = []
        for inst in orig:
            if id(inst) in spill_nops:
                new_list.extend(spill_nops[id(inst)])
            new_list.append(inst)
        # rebuilding from `orig` also drops any freshly created nops that
        # bass appended to this block's tail
        blk.instructions[:] = new_list
    return n_fixed


def get_nc():
    if "nc" not in _NC_CACHE:
        _ensure_paths()
        _NC_CACHE["nc"] = _build_nc()
    return _NC_CACHE["nc"]


def kernel(**inputs):
    _ensure_paths()
    from concourse.bass_utils import run_bass_kernel_spmd

    nc = get_nc()
    in_maps = build_in_maps(inputs)
    res = run_bass_kernel_spmd(nc, in_maps, list(range(NCORES)))
    out = np.concatenate(
        [decode_out(res.results[c]["output"]) for c in range(NCORES)], axis=0
    )
    return out.astype(np.float32)


# revision 5
# speedup vs baseline: 1.5312x; 1.0305x over previous
"""GRU cell kernel for Trainium2, data-parallel across 8 NeuronCores.

Per core: batch shard of 1024 rows; weights replicated.
  u  = sigmoid(x @ Wxu + h @ Whu + bu)
  r  = sigmoid(x @ Wxr + h @ Whr + br)
  c' = tanh  (x @ Wxc + (h*r) @ Whc + bc)
  c  = u*c' + (1-u)*h  =  u*c' - (u-1)*h

All layout work happens on the HOST: activations are fed pre-transposed
([feature, batch]) and pre-packed in the exact SBUF column order, weights
pre-converted to bf16 in stationary-operand order, and the output comes
back feature-major/bf16 and is unscrambled host-side.  The device runs
nothing but the 768 bf16 matmuls (128x128x512 each), the activations and
the blend -- no on-chip transposes, no dtype casts.

Gate r's first batch-half is phase-split (all 8 output chunks' x-side
matmuls first, h-sides after) so the PE has dense work while the h/Whr
DMAs stream in.  (u-1)*h is precomputed during the u phase so the
candidate tail is only tanh -> mul -> sub -> DMA.

SBUF column layouts (all tiles [128 partitions x cols]):
  activations x,h : col = n*4096 + k*512 + b   (n: batch half, k: feat chunk)
  weights         : col = j*1024 + k*128 + jc  (j: out chunk, k: contraction)
  output          : col = j*1024 + n*512 + b
"""

import os
import sys

import numpy as np

B = 8192
E = 1024
H = 1024
NCORES = 8
B_SH = B // NCORES  # 1024 rows per core

P = 128
KE = E // P   # 8 contraction chunks
NJ = H // P   # 8 output feature chunks
BN = 512      # moving free-dim per matmul
NB = B_SH // BN  # 2

W_NAMES = ("Wxu", "Whu", "Wxr", "Whr", "Wxc", "Whc")
B_NAMES = ("bu", "br", "bc")

_NC_CACHE = {}


def _ensure_paths():
    for p in ("/opt/trn_rl_repo", "/root/.axon_site/_ro/trn_rl_repo"):
        if os.path.isdir(p) and p not in sys.path:
            sys.path.insert(0, p)


def _bf16():
    import ml_dtypes

    return ml_dtypes.bfloat16


def pack_act(a):
    """[B_SH, 1024] f32 -> [128, 8192] bf16, col = n*4096 + k*512 + b."""
    t = np.asarray(a, np.float32).astype(_bf16()).T          # [feat, batch]
    t = t.reshape(KE, P, NB, BN)                             # [k, p, n, b]
    return np.ascontiguousarray(t.transpose(1, 2, 0, 3).reshape(P, NB * KE * BN))


def pack_w(w):
    """[1024, 1024] f32 -> [128, 8192] bf16, col = j*1024 + k*128 + jc."""
    t = np.asarray(w, np.float32).astype(_bf16()).reshape(KE, P, NJ, P)
    return np.ascontiguousarray(t.transpose(1, 2, 0, 3).reshape(P, NJ * KE * P))


def pack_bias(b):
    """[1, 1024] f32 -> [128, 8] f32, col j holds features j*128..j*128+127."""
    return np.ascontiguousarray(
        np.asarray(b, np.float32).reshape(NJ, P).T.astype(np.float32)
    )


def decode_out(o):
    """[128, 8192] bf16 (p; j,n,b cols) -> [B_SH, 1024] f32 [batch, feature]."""
    return np.ascontiguousarray(
        np.asarray(o)
        .astype(np.float32)
        .reshape(P, NJ, NB, BN)
        .transpose(2, 3, 1, 0)
        .reshape(B_SH, H)
    )


def build_in_maps(inputs):
    x = np.asarray(inputs["input"], dtype=np.float32)
    h = np.asarray(inputs["hidden_state"], dtype=np.float32)
    shared = {n: pack_w(inputs[n]) for n in W_NAMES}
    shared.update({n: pack_bias(inputs[n]) for n in B_NAMES})
    in_maps = []
    for c in range(NCORES):
        m = {
            "input": pack_act(x[c * B_SH : (c + 1) * B_SH]),
            "hidden_state": pack_act(h[c * B_SH : (c + 1) * B_SH]),
        }
        m.update(shared)
        in_maps.append(m)
    return in_maps


def _build_nc():
    import concourse.bass as bass
    import concourse.mybir as mybir
    from concourse.tile import TileContext

    f32 = mybir.dt.float32
    bf16 = mybir.dt.bfloat16
    AF = mybir.ActivationFunctionType
    ALU = mybir.AluOpType

    ACT_COLS = NB * KE * BN  # 8192
    W_COLS = NJ * KE * P     # 8192

    nc = bass.Bass()
    x_d = nc.dram_tensor("input", [P, ACT_COLS], bf16, kind="ExternalInput")
    h_d = nc.dram_tensor("hidden_state", [P, ACT_COLS], bf16, kind="ExternalInput")
    w_d = {n: nc.dram_tensor(n, [P, W_COLS], bf16, kind="ExternalInput") for n in W_NAMES}
    b_d = {n: nc.dram_tensor(n, [P, NJ], f32, kind="ExternalInput") for n in B_NAMES}
    out_d = nc.dram_tensor("output", [P, NJ * B_SH], bf16, kind="ExternalOutput")

    with TileContext(nc) as tc:
        with (
            tc.tile_pool(name="sb", bufs=1) as sb,
            tc.tile_pool(name="psum", bufs=1, space="PSUM") as pp,
        ):
            xs = sb.tile([P, ACT_COLS], bf16, tag="xs", bufs=1)
            hs = sb.tile([P, ACT_COLS], bf16, tag="hs", bufs=1)
            us = sb.tile([P, ACT_COLS], bf16, tag="us", bufs=1)
            up = sb.tile([P, ACT_COLS], bf16, tag="up", bufs=1)  # (u-1)*h
            rhs = sb.tile([P, ACT_COLS], bf16, tag="rhs", bufs=1)
            ws = {
                n: sb.tile([P, W_COLS], bf16, tag=f"w_{n}", name=f"w_{n}", bufs=1)
                for n in W_NAMES
            }
            bt = {
                n: sb.tile([P, NJ], f32, tag=f"b_{n}", name=f"b_{n}", bufs=1)
                for n in B_NAMES
            }

            Q = ACT_COLS // 4  # 2048
            HALF = ACT_COLS // 2

            # ---- input DMAs ordered to feed gate r's phase-split start
            nc.sync.dma_start(ws["Wxr"][:, 0:Q], w_d["Wxr"][:, 0:Q])        # j0,j1
            nc.sync.dma_start(xs[:, 0:Q], x_d[:, 0:Q])                      # n0 k0-3
            nc.sync.dma_start(ws["Wxr"][:, Q:HALF], w_d["Wxr"][:, Q:HALF])  # j2,j3
            nc.sync.dma_start(xs[:, Q:HALF], x_d[:, Q:HALF])                # n0 k4-7
            nc.sync.dma_start(ws["Wxr"][:, HALF:], w_d["Wxr"][:, HALF:])    # j4-7
            nc.sync.dma_start(ws["Whr"][:, 0:HALF], w_d["Whr"][:, 0:HALF])  # j0-3
            nc.sync.dma_start(hs[:, 0:HALF], h_d[:, 0:HALF])                # n0
            nc.sync.dma_start(ws["Whr"][:, HALF:], w_d["Whr"][:, HALF:])    # j4-7
            nc.sync.dma_start(xs[:, HALF:], x_d[:, HALF:])                  # n1
            nc.sync.dma_start(hs[:, HALF:], h_d[:, HALF:])                  # n1
            for n in B_NAMES:
                nc.sync.dma_start(bt[n][:, :], b_d[n][:, :])
            for wn in ("Wxu", "Whu", "Wxc", "Whc"):
                nc.sync.dma_start(ws[wn][:, 0:HALF], w_d[wn][:, 0:HALF])
                nc.sync.dma_start(ws[wn][:, HALF:], w_d[wn][:, HALF:])

            def xside(ps, wx, j, n, stop=False):
                for k in range(KE):
                    nc.tensor.matmul(
                        ps[:],
                        wx[:, j * (KE * P) + k * P : j * (KE * P) + (k + 1) * P],
                        xs[:, n * (KE * BN) + k * BN : n * (KE * BN) + (k + 1) * BN],
                        start=(k == 0),
                        stop=False,
                    )

            def hside(ps, wh, rhsT, j, n):
                for k in range(KE):
                    nc.tensor.matmul(
                        ps[:],
                        wh[:, j * (KE * P) + k * P : j * (KE * P) + (k + 1) * P],
                        rhsT[:, n * (KE * BN) + k * BN : n * (KE * BN) + (k + 1) * BN],
                        start=False,
                        stop=(k == KE - 1),
                    )

            def act_slice(t, j, n):
                return t[:, n * (KE * BN) + j * BN : n * (KE * BN) + (j + 1) * BN]

            def fold_r(ps, j, n):
                rt = sb.tile([P, BN], bf16, tag="rt", bufs=4)
                nc.scalar.activation(
                    rt[:], ps[:], AF.Sigmoid, bias=bt["br"][:, j : j + 1]
                )
                nc.vector.tensor_mul(act_slice(rhs, j, n), rt[:], act_slice(hs, j, n))

            # ---- gate r, n=0: phase-split so PE works while h/Whr stream in
            ps_r0 = [
                pp.tile([P, BN], f32, tag="mm", name=f"ps_r0_{j}", bufs=8)
                for j in range(NJ)
            ]
            for j in range(NJ):
                xside(ps_r0[j], ws["Wxr"], j, 0)
            for j in range(NJ):
                hside(ps_r0[j], ws["Whr"], hs, j, 0)
                fold_r(ps_r0[j], j, 0)

            # ---- gate r, n=1: contiguous groups
            for j in range(NJ):
                ps = pp.tile([P, BN], f32, tag="mm", bufs=8)
                xside(ps, ws["Wxr"], j, 1)
                hside(ps, ws["Whr"], hs, j, 1)
                fold_r(ps, j, 1)

            # ---- gate u: sigmoid -> us, and up = (u-1)*h for the blend
            for j in range(NJ):
                for n in range(NB):
                    ps = pp.tile([P, BN], f32, tag="mm", bufs=8)
                    xside(ps, ws["Wxu"], j, n)
                    hside(ps, ws["Whu"], hs, j, n)
                    nc.scalar.activation(
                        act_slice(us, j, n), ps[:], AF.Sigmoid,
                        bias=bt["bu"][:, j : j + 1],
                    )
                    nc.vector.scalar_tensor_tensor(
                        act_slice(up, j, n),
                        act_slice(us, j, n),
                        1.0,
                        act_slice(hs, j, n),
                        ALU.subtract,
                        ALU.mult,
                    )

            # ---- candidate + blend + store:  c = u*c' - (u-1)*h
            for j in range(NJ):
                for n in range(NB):
                    ps = pp.tile([P, BN], f32, tag="mm", bufs=8)
                    xside(ps, ws["Wxc"], j, n)
                    hside(ps, ws["Whc"], rhs, j, n)
                    cc = sb.tile([P, BN], bf16, tag="cc", bufs=4)
                    nc.scalar.activation(
                        cc[:], ps[:], AF.Tanh, bias=bt["bc"][:, j : j + 1]
                    )
                    nc.vector.tensor_mul(cc[:], cc[:], act_slice(us, j, n))
                    nc.vector.tensor_sub(cc[:], cc[:], act_slice(up, j, n))
                    nc.gpsimd.dma_start(
                        out_d[:, j * B_SH + n * BN : j * B_SH + (n + 1) * BN], cc[:]
                    )

    _split_matmul_waits(nc, mybir)
    return nc


def _split_matmul_waits(nc, mybir):
    """Walrus codegen allows only one sync-wait on a Matmult (it lowers to an
    LDW+MM pair).  Spill extra waits onto a PE NoOp placed just before."""
    n_fixed = 0
    blocks = list(nc.m.functions[0].blocks)
    origs = [list(b.instructions) for b in blocks]
    spill_nops = {}  # id(inst) -> [nop insts]
    for orig in origs:
        for inst in orig:
            si = inst.sync_info
            if (
                si is not None
                and si.on_wait
                and len(si.on_wait) > 1
            ):
                waits = list(si.on_wait)
                eng = nc.engines[inst.engine]
                nops = []
                for w in waits[:-1]:
                    nop = eng.nop(hint="waitspill").ins
                    nop.sync_info = mybir.SyncInfo(on_wait=[w], on_update=[])
                    nops.append(nop)
                inst.sync_info = mybir.SyncInfo(
                    on_wait=waits[-1:], on_update=list(si.on_update or [])
                )
                spill_nops[id(inst)] = nops
                n_fixed += 1
    for blk, orig in zip(blocks, origs):
        new_list = []
        for inst in orig:
            if id(inst) in spill_nops:
                new_list.extend(spill_nops[id(inst)])
            new_list.append(inst)
        # rebuilding from `orig` also drops any freshly created nops that
        # bass appended to this block's tail
        blk.instructions[:] = new_list
    return n_fixed


def get_nc():
    if "nc" not in _NC_CACHE:
        _ensure_paths()
        _NC_CACHE["nc"] = _build_nc()
    return _NC_CACHE["nc"]


def kernel(**inputs):
    _ensure_paths()
    from concourse.bass_utils import run_bass_kernel_spmd

    nc = get_nc()
    in_maps = build_in_maps(inputs)
    res = run_bass_kernel_spmd(nc, in_maps, list(range(NCORES)))
    out = np.concatenate(
        [decode_out(res.results[c]["output"]) for c in range(NCORES)], axis=0
    )
    return out.astype(np.float32)
